# revision 6
# baseline (speedup 1.0000x reference)
"""BitLinear-1.58 (absmean ternary quant + linear) on 8 TRN2 NeuronCores.

Problem: x[4, 2048, 4096] f32, weight[16384, 4096] f32, bias[16384] f32.
    w_q = sign(w) * (|w| >= 0.7 * mean(|w|))   (global mean over all of w)
    y   = x @ w_q.T + bias                      -> [4, 2048, 16384] f32

Sharding (column/tensor parallel): weight & bias are sharded along
out_features across the 8 cores (2048 rows each); x is replicated.
Each core computes y_shard [8192, 2048]; the host concatenates shards.

Per-core device program:
  phase A: partial = sum(|w_shard|) via DVE reduce; PE ones-matmul folds
           partitions; 8-core AllReduce of the scalar -> global sum ->
           threshold thr = 0.7 * gsum * 2^-26 (2^26 = weight element count).
  phase B: ternary quant in f32 (exact mask semantics vs the reference):
           wq = (w >= thr) - (w <= -thr), stored bf16, fully resident in
           SBUF (32 k-tiles of [128, 2048] = 128 KB/partition).
  phase C: y^T tiles: for each 128-token tile, accumulate 32 k-tile
           matmuls (lhsT = x^T k-tile [128k, 128t] bf16 stationary,
           rhs = wq k-tile [128k, 512o]) into 4 PSUM banks, add bias,
           DMA out.

x is fed pre-transposed/cast on the host (x^T bf16 [4096, 8192],
replicated to all cores) so both matmul operands have the contraction
dim on partitions with DMA-friendly layouts. The quant mask itself is
computed from the original f32 weights on-device.
"""

import numpy as np
import ml_dtypes

import concourse.bacc as bacc
import concourse.mybir as mybir
import concourse.tile as tile
import concourse.bass_utils as bass_utils

F32 = mybir.dt.float32
BF16 = mybir.dt.bfloat16
ALU = mybir.AluOpType
AX = mybir.AxisListType

N_CORES = 8
B, S, K, O_TOTAL = 4, 2048, 4096, 16384
T = B * S                  # 8192 tokens
O = O_TOTAL // N_CORES     # 2048 out features per core
KT = K // 128              # 32 k-tiles
N_OC = O // 512            # 4 output chunks of 512
T_SUPER = 128              # tokens per x DMA super-tile
N_TSUP = T // T_SUPER      # 32
INV_N = 1.0 / (O_TOTAL * K)  # 2^-26, exact power of two

_NC_CACHE = None


def build_nc():
    nc = bacc.Bacc("TRN2", target_bir_lowering=False, debug=False,
                   num_devices=N_CORES)
    xT = nc.dram_tensor("xT", [K, T], BF16, kind="ExternalInput")
    wT = nc.dram_tensor("wT", [K, O], F32, kind="ExternalInput")
    bias = nc.dram_tensor("bias", [1, O], F32, kind="ExternalInput")
    y = nc.dram_tensor("y", [T, O], F32, kind="ExternalOutput")

    with tile.TileContext(nc) as tc:
        with (
            tc.tile_pool(name="wf", bufs=3) as wf,        # f32 weight staging
            tc.tile_pool(name="mf", bufs=2) as mf,        # quant mask staging
            tc.tile_pool(name="wqp", bufs=KT) as wqp,     # resident ternary w
            tc.tile_pool(name="xp", bufs=2) as xp,        # x^T staging
            tc.tile_pool(name="op", bufs=2) as op,        # output staging
            tc.tile_pool(name="small", bufs=1) as small,
            tc.tile_pool(name="psum", bufs=8, space="PSUM") as psum,
            tc.tile_pool(name="dram", bufs=1, space="DRAM") as dram,
        ):
            # w DMAs alternate between two DGE queues (sync + scalar) so
            # two 1MB transfers overlap (~2x single-queue bandwidth).
            w_dma_engines = [nc.sync, nc.scalar]

            # ---------------- phase A: global absmean threshold ----------
            with nc.named_scope("scaleA"):
                partials = small.tile([128, KT], F32)
                for i in range(KT):
                    wt = wf.tile([128, O], F32, tag="w", name=f"wa_{i}")
                    w_dma_engines[i % 2].dma_start(
                        wt[:], wT[i * 128:(i + 1) * 128, :])
                    nc.vector.tensor_reduce(
                        partials[:, i:i + 1], wt[:], AX.X, ALU.add,
                        apply_absolute_value=True)

                col = small.tile([128, 1], F32)
                nc.vector.tensor_reduce(col[:], partials[:], AX.X, ALU.add)
                ones = small.tile([128, 1], F32)
                nc.any.memset(ones[:], 1.0)
                ps_scalar = psum.tile([1, 1], F32, tag="acc")
                nc.tensor.matmul(ps_scalar[:], ones[:], col[:])
                local_sum = small.tile([1, 1], F32)
                nc.vector.tensor_copy(local_sum[:], ps_scalar[:])

                in_b = dram.tile([1, 1], F32)
                out_b = dram.tile([1, 1], F32)
                nc.gpsimd.dma_start(in_b[:], local_sum[:])
                nc.gpsimd.collective_compute(
                    "AllReduce", ALU.add,
                    replica_groups=[list(range(N_CORES))],
                    ins=[in_b[:]], outs=[out_b[:]])
                gsum = small.tile([1, 1], F32)
                nc.gpsimd.dma_start(gsum[:], out_b[:])

            # bias: broadcast-DMA the [1, O] row to all 128 partitions
            bias_sb = small.tile([128, O], F32)
            nc.gpsimd.dma_start(bias_sb[:], bias.ap().to_broadcast((128, O)))

            # phase B weight reloads: issued right after pass A so the
            # first few stream during the collective.
            w_tiles = []
            for i in range(KT):
                wt = wf.tile([128, O], F32, tag="w", name=f"wb_{i}")
                w_dma_engines[i % 2].dma_start(
                    wt[:], wT[i * 128:(i + 1) * 128, :])
                w_tiles.append(wt)

            # thr = (gsum * 2^-26) * 0.7 ; matches reference rounding
            thr1 = small.tile([1, 1], F32)
            nc.vector.tensor_scalar(thr1[:], gsum[:], INV_N, 0.7,
                                    ALU.mult, ALU.mult)
            thr = small.tile([128, 1], F32)
            nc.gpsimd.partition_broadcast(thr[:], thr1[:])
            nthr = small.tile([128, 1], F32)
            nc.vector.tensor_scalar_mul(nthr[:], thr[:], -1.0)

            # ---------------- phase B: ternary quant ---------------------
            # wq = (w >= thr) - (w <= -thr), f32 compares, bf16 result.
            # mneg on GpSimd, fused compare-subtract on DVE (parallel).
            wq = []
            with nc.named_scope("quantB"):
                for i in range(KT):
                    wt = w_tiles[i]
                    mneg = mf.tile([128, O], BF16, tag="mneg")
                    nc.gpsimd.tensor_scalar(mneg[:], wt[:], nthr[:], None,
                                            ALU.is_le)
                    wqt = wqp.tile([128, O], BF16, tag="wq")
                    nc.vector.scalar_tensor_tensor(
                        wqt[:], wt[:], thr[:], mneg[:],
                        ALU.is_ge, ALU.subtract)
                    wq.append(wqt)

            # ---------------- phase C: matmul + bias ---------------------
            # oc-major chains: 32 consecutive MMs accumulate into the SAME
            # PSUM bank (bank switches between MMs cost ~47ns each).
            xT_r = xT.ap().rearrange("(kt p) t -> p kt t", p=128)
            with nc.named_scope("matmulC"):
                for tsup in range(N_TSUP):
                    x_sb = xp.tile([128, KT, T_SUPER], BF16, tag="x")
                    nc.sync.dma_start(
                        x_sb[:],
                        xT_r[:, :, tsup * T_SUPER:(tsup + 1) * T_SUPER])
                    for sub in range(T_SUPER // 128):
                        t0 = tsup * T_SUPER + sub * 128
                        xk = [x_sb[:, k, sub * 128:(sub + 1) * 128]
                              for k in range(KT)]
                        out_sb = op.tile([128, O], F32, tag="out")
                        for oc in range(N_OC):
                            acc = psum.tile([128, 512], F32, tag="acc",
                                            name=f"acc_{t0}_{oc}")
                            for k in range(KT):
                                nc.tensor.matmul(
                                    acc[:], xk[k],
                                    wq[k][:, oc * 512:(oc + 1) * 512],
                                    start=(k == 0), stop=(k == KT - 1))
                            nc.vector.tensor_tensor(
                                out_sb[:, oc * 512:(oc + 1) * 512],
                                acc[:],
                                bias_sb[:, oc * 512:(oc + 1) * 512],
                                ALU.add)
                        nc.scalar.dma_start(y[t0:t0 + 128, :], out_sb[:])

    nc.compile()
    return nc


def get_nc():
    global _NC_CACHE
    if _NC_CACHE is None:
        _NC_CACHE = build_nc()
    return _NC_CACHE


def prep_in_maps(x: np.ndarray, weight: np.ndarray, bias: np.ndarray):
    """Host-side sharding/layout: transpose + bf16-cast x (replicated),
    shard weight/bias along out_features."""
    xT = np.ascontiguousarray(x.reshape(T, K).T).astype(ml_dtypes.bfloat16)
    wT_full = weight.T  # [K, O_TOTAL] view
    in_maps = []
    for c in range(N_CORES):
        in_maps.append({
            "xT": xT,
            "wT": np.ascontiguousarray(wT_full[:, c * O:(c + 1) * O]),
            "bias": np.ascontiguousarray(
                bias[c * O:(c + 1) * O].reshape(1, O)).astype(np.float32),
        })
    return in_maps


def run_shards(in_maps, trace=False):
    nc = get_nc()
    return bass_utils.run_bass_kernel_spmd(
        nc, in_maps, core_ids=list(range(N_CORES)), trace=trace)


def kernel(x: np.ndarray, weight: np.ndarray, bias: np.ndarray) -> np.ndarray:
    x = np.asarray(x, dtype=np.float32)
    weight = np.asarray(weight, dtype=np.float32)
    bias = np.asarray(bias, dtype=np.float32)
    res = run_shards(prep_in_maps(x, weight, bias), trace=False)
    y = np.concatenate([res.results[c]["y"] for c in range(N_CORES)], axis=1)
    return y.reshape(B, S, O_TOTAL)


# revision 9
# speedup vs baseline: 1.3668x; 1.3668x over previous
"""BitLinear-1.58 (absmean ternary quant + linear) on 8 TRN2 NeuronCores.

Problem: x[4, 2048, 4096] f32, weight[16384, 4096] f32, bias[16384] f32.
    w_q = sign(w) * (|w| >= 0.7 * mean(|w|))   (global mean over all of w)
    y   = x @ w_q.T + bias                      -> [4, 2048, 16384] f32

Sharding (column/tensor parallel): weight & bias sharded along
out_features across 8 cores (2048 each); x replicated. Each core
computes y_shard [8192, 2048]; the host concatenates shards.

Per-core device program:
  A: local sum(|w_shard|) (DVE reduce over 3-queue DMA stream), PE
     ones-matmul partition fold, 8-core AllReduce of the scalar,
     thr = 0.7 * gsum * 2^-26.
  B: ternary quant, oc-major: wq = (w >= thr) - (w <= -thr) computed in
     f32 (exact reference mask semantics), stored as 128 resident
     [128k, 512o] bf16 tiles (32 k-tiles x 4 o-chunks = 128 KB/part).
  C: matmul: chains of 32 accumulating MMs (lhsT = x^T k-slice
     [128k,128t] bf16, rhs = wq tile [128k,512o]) into one PSUM bank,
     + bias, streamed over 64 token tiles. The first 4 token tiles are
     emitted oc-major ("strip") so the PE starts right after the first
     quantized o-chunk instead of after the whole quant phase.

Note: with all 8 cores saturated the chip power-throttles the PE to
~1.95 GHz (k=13/16), so the per-MM floor is ~263 ns, not 216.

x is fed pre-transposed/cast on the host (x^T bf16 [4096, 8192],
replicated) so both matmul operands have the contraction dim on
partitions with DMA-friendly layouts.
"""

import numpy as np
import ml_dtypes

import concourse.bacc as bacc
import concourse.mybir as mybir
import concourse.tile as tile
import concourse.bass_utils as bass_utils

F32 = mybir.dt.float32
BF16 = mybir.dt.bfloat16
ALU = mybir.AluOpType
AX = mybir.AxisListType

N_CORES = 8
B, S, K, O_TOTAL = 4, 2048, 4096, 16384
T = B * S                  # 8192 tokens
O = O_TOTAL // N_CORES     # 2048 out features per core
KT = K // 128              # 32 k-tiles
N_OC = O // 512            # 4 output chunks of 512
NT = T // 128              # 64 token tiles
STRIP = 4                  # leading token tiles emitted oc-major
INV_N = 1.0 / (O_TOTAL * K)  # 2^-26, exact power of two

_NC_CACHE = {}


def build_nc(with_bias: bool):
    nc = bacc.Bacc("TRN2", target_bir_lowering=False, debug=False,
                   num_devices=N_CORES)
    xT = nc.dram_tensor("xT", [K, T], BF16, kind="ExternalInput")
    wT = nc.dram_tensor("wT", [K, O], F32, kind="ExternalInput")
    bias = nc.dram_tensor("bias", [1, O], F32, kind="ExternalInput")
    y = nc.dram_tensor("y", [T, O], F32, kind="ExternalOutput")

    with tile.TileContext(nc) as tc:
        with (
            tc.tile_pool(name="wf", bufs=4) as wf,        # pass A staging
            tc.tile_pool(name="wb", bufs=4) as wb,        # pass B staging
            tc.tile_pool(name="mf", bufs=2) as mf,        # quant mask
            tc.tile_pool(name="wqp", bufs=KT * N_OC) as wqp,  # ternary w
            tc.tile_pool(name="xp", bufs=STRIP) as xp,    # x^T staging
            tc.tile_pool(name="op", bufs=8) as op,        # out staging
            tc.tile_pool(name="small", bufs=1) as small,
            tc.tile_pool(name="psum", bufs=8, space="PSUM") as psum,
            tc.tile_pool(name="dram", bufs=1, space="DRAM") as dram,
        ):
            # ---------------- phase A: global absmean threshold ----------
            # 64 half-k-tile slices [128, 1024] f32 over three DGE queues.
            NA = 2 * KT
            a_engines = [nc.sync, nc.scalar, nc.gpsimd]
            with nc.named_scope("scaleA"):
                partials = small.tile([128, NA], F32)
                for i in range(NA):
                    kt, h = divmod(i, 2)
                    wt = wf.tile([128, 1024], F32, tag="w", name=f"wa_{i}")
                    a_engines[i % 3].dma_start(
                        wt[:], wT[kt * 128:(kt + 1) * 128,
                                  h * 1024:(h + 1) * 1024])
                    nc.vector.tensor_reduce(
                        partials[:, i:i + 1], wt[:], AX.X, ALU.add,
                        apply_absolute_value=True)

                col = small.tile([128, 1], F32)
                nc.vector.tensor_reduce(col[:], partials[:], AX.X, ALU.add)
                ones = small.tile([128, 1], F32)
                nc.any.memset(ones[:], 1.0)
                ps_scalar = psum.tile([1, 1], F32, tag="acc")
                nc.tensor.matmul(ps_scalar[:], ones[:], col[:])
                local_sum = small.tile([1, 1], F32)
                nc.vector.tensor_copy(local_sum[:], ps_scalar[:])

                in_b = dram.tile([1, 1], F32)
                out_b = dram.tile([1, 1], F32)
                nc.gpsimd.dma_start(in_b[:], local_sum[:])
                nc.gpsimd.collective_compute(
                    "AllReduce", ALU.add,
                    replica_groups=[list(range(N_CORES))],
                    ins=[in_b[:]], outs=[out_b[:]])
                gsum = small.tile([1, 1], F32)
                nc.gpsimd.dma_start(gsum[:], out_b[:])

            if with_bias:
                bias_sb = small.tile([128, O], F32)
                nc.gpsimd.dma_start(bias_sb[:],
                                    bias.ap().to_broadcast((128, O)))

            # thr = (gsum * 2^-26) * 0.7 ; matches reference rounding
            thr1 = small.tile([1, 1], F32)
            nc.vector.tensor_scalar(thr1[:], gsum[:], INV_N, 0.7,
                                    ALU.mult, ALU.mult)
            thr = small.tile([128, 1], F32)
            nc.gpsimd.partition_broadcast(thr[:], thr1[:])
            nthr = small.tile([128, 1], F32)
            nc.vector.tensor_scalar_mul(nthr[:], thr[:], -1.0)

            # x^T prefetch for the strip tiles, on the gpsimd queue so it
            # doesn't sit behind the phase-B weight stream.
            xT_r = xT.ap().rearrange("(kt p) t -> p kt t", p=128)
            x_tiles = {}
            for t in range(STRIP):
                x_sb = xp.tile([128, KT, 128], BF16, tag="x",
                               name=f"x_{t}")
                nc.gpsimd.dma_start(
                    x_sb[:], xT_r[:, :, t * 128:(t + 1) * 128])
                x_tiles[t] = x_sb

            # phase B weight reloads, oc-major [128, 512] f32 slices on
            # two queues; the first o-chunk's 32 slices arrive first.
            wb_tiles = {}
            b_engines = [nc.sync, nc.scalar]
            for oc in range(N_OC):
                for k in range(KT):
                    wt = wb.tile([128, 512], F32, tag="wb",
                                 name=f"wb_{oc}_{k}")
                    b_engines[k % 2].dma_start(
                        wt[:], wT[k * 128:(k + 1) * 128,
                                  oc * 512:(oc + 1) * 512])
                    wb_tiles[(oc, k)] = wt

            # ---------------- phase B: ternary quant (oc-major) ----------
            # wq = (w >= thr) - (w <= -thr); f32 compares, bf16 result
            wq = {}
            with nc.named_scope("quantB"):
                for oc in range(N_OC):
                    for k in range(KT):
                        wt = wb_tiles[(oc, k)]
                        mneg = mf.tile([128, 512], BF16, tag="mneg")
                        nc.vector.tensor_scalar(
                            mneg[:], wt[:], nthr[:], None, ALU.is_le)
                        wqt = wqp.tile([128, 512], BF16, tag="wq",
                                       name=f"wq_{oc}_{k}")
                        nc.vector.scalar_tensor_tensor(
                            wqt[:], wt[:], thr[:], mneg[:],
                            ALU.is_ge, ALU.subtract)
                        wq[(oc, k)] = wqt

            # ---------------- phase C: matmul + bias ---------------------
            def chain(t, oc, ep_engine):
                """One 32-MM accumulation chain + epilogue + y DMA."""
                x_sb = x_tiles[t]
                acc = psum.tile([128, 512], F32, tag="acc",
                                name=f"acc_{t}_{oc}")
                for k in range(KT):
                    nc.tensor.matmul(acc[:], x_sb[:, k, :], wq[(oc, k)][:],
                                     start=(k == 0), stop=(k == KT - 1))
                out_sb = op.tile([128, 512], F32, tag="out",
                                 name=f"o_{t}_{oc}")
                if with_bias:
                    nc.vector.tensor_tensor(
                        out_sb[:], acc[:],
                        bias_sb[:, oc * 512:(oc + 1) * 512], ALU.add)
                elif ep_engine == 0:
                    nc.vector.tensor_copy(out_sb[:], acc[:])
                else:
                    nc.scalar.copy(out_sb[:], acc[:])
                nc.gpsimd.dma_start(
                    y[t * 128:(t + 1) * 128, oc * 512:(oc + 1) * 512],
                    out_sb[:])

            with nc.named_scope("matmulC"):
                ep = 0
                # strip: oc-major over the first STRIP token tiles
                for oc in range(N_OC):
                    for t in range(STRIP):
                        chain(t, oc, ep)
                        ep ^= 1
                # steady state: token-major
                for t in range(STRIP, NT):
                    x_sb = xp.tile([128, KT, 128], BF16, tag="x",
                                   name=f"x_{t}")
                    nc.sync.dma_start(
                        x_sb[:], xT_r[:, :, t * 128:(t + 1) * 128])
                    x_tiles[t] = x_sb
                    for oc in range(N_OC):
                        chain(t, oc, ep)
                        ep ^= 1

    nc.compile()
    return nc


def get_nc(with_bias: bool):
    if with_bias not in _NC_CACHE:
        _NC_CACHE[with_bias] = build_nc(with_bias)
    return _NC_CACHE[with_bias]


def prep_in_maps(x: np.ndarray, weight: np.ndarray, bias: np.ndarray):
    """Host-side sharding/layout: transpose + bf16-cast x (replicated),
    shard weight/bias along out_features."""
    xT = np.ascontiguousarray(x.reshape(T, K).T).astype(ml_dtypes.bfloat16)
    wT_full = weight.T  # [K, O_TOTAL] view
    in_maps = []
    for c in range(N_CORES):
        in_maps.append({
            "xT": xT,
            "wT": np.ascontiguousarray(wT_full[:, c * O:(c + 1) * O]),
            "bias": np.ascontiguousarray(
                bias[c * O:(c + 1) * O].reshape(1, O)).astype(np.float32),
        })
    return in_maps


def run_shards(in_maps, trace=False, with_bias=None):
    if with_bias is None:
        with_bias = any(np.any(m["bias"]) for m in in_maps)
    nc = get_nc(with_bias)
    return bass_utils.run_bass_kernel_spmd(
        nc, in_maps, core_ids=list(range(N_CORES)), trace=trace)


def kernel(x: np.ndarray, weight: np.ndarray, bias: np.ndarray) -> np.ndarray:
    x = np.asarray(x, dtype=np.float32)
    weight = np.asarray(weight, dtype=np.float32)
    bias = np.asarray(bias, dtype=np.float32)
    res = run_shards(prep_in_maps(x, weight, bias))
    y = np.concatenate([res.results[c]["y"] for c in range(N_CORES)], axis=1)
    return y.reshape(B, S, O_TOTAL)
